# revision 24
# baseline (speedup 1.0000x reference)
"""Trainium2 Bass kernel for nn_CausalGemmaMiniBitterLLM (segment_reduce).

Self-contained: hardcodes shapes B=4, S=2048, D=1024, V=32000, 8 cores.
Sharding: every core runs the (cheap) merge pipeline for the full batch and
computes a column shard (V/8 = 4000) of the vocab projection; log_softmax's
cross-vocab sum is combined with an 8-core AllReduce on-device.

kernel(**inputs) -> tuple matching reference.reference(**inputs).
"""
import numpy as np

import concourse.bass as bass
import concourse.bacc as bacc
import concourse.mybir as mybir
import concourse.tile as tile
from concourse import bass2jax

dt = mybir.dt
AF = mybir.ActivationFunctionType
ALU = mybir.AluOpType
IOff = bass.IndirectOffsetOnAxis

P = 128


def full_cfg():
    return dict(B=4, S=2048, D=1024, V=32000, NCORES=8, GROUP=8)


def small_cfg():
    # scaled config for fast CoreSim validation
    return dict(B=2, S=256, D=256, V=2048, NCORES=8, GROUP=2)


def _derived(cfg):
    c = dict(cfg)
    c["SC"] = c["S"] // P            # chunks per row
    c["NCH"] = c["B"] * c["SC"]      # total chunks (= total token tiles)
    c["DC"] = c["D"] // P            # d chunks
    c["VS"] = c["V"] // c["NCORES"]  # vocab shard width
    c["AUGW"] = c["D"] + 64          # pooled row: D, pos, cnt, pad (256B mult)
    # vocab tiles of <=512, bank aligned
    vt = []
    off = 0
    while off < c["VS"]:
        n = min(512, c["VS"] - off)
        vt.append((off, n))
        off += n
    c["VT"] = vt
    assert c["NCH"] % c["GROUP"] == 0
    return c


def build_nc(cfg, with_collective=True):
    c = _derived(cfg)
    B, S, D, V = c["B"], c["S"], c["D"], c["V"]
    SC, NCH, DC, VS, AUGW = c["SC"], c["NCH"], c["DC"], c["VS"], c["AUGW"]
    G = c["GROUP"]
    NCORES = c["NCORES"]
    f32, bf16, i32 = dt.float32, dt.bfloat16, dt.int32

    nc = bacc.Bacc("TRN2", target_bir_lowering=False, debug=False,
                   num_devices=NCORES)

    # ---------------- I/O ----------------
    ids_d = nc.dram_tensor("input_ids", [B, S], i32, kind="ExternalInput")
    noise_d = nc.dram_tensor("gate_noise", [B, S], f32, kind="ExternalInput")
    emb_d = nc.dram_tensor("emb", [V, D], f32, kind="ExternalInput")
    gwb_d = nc.dram_tensor("gate_wb", [P, AUGW], f32, kind="ExternalInput")
    wsh_d = nc.dram_tensor("w_shard", [DC, P, VS], bf16, kind="ExternalInput")
    bias_d = nc.dram_tensor("bias_bc", [P, VS], bf16, kind="ExternalInput")
    tri_d = nc.dram_tensor("tri128", [P, P], f32, kind="ExternalInput")
    tris_d = nc.dram_tensor("tri_chunk", [NCH, NCH], f32, kind="ExternalInput")
    ident_d = nc.dram_tensor("ident", [P, P], f32, kind="ExternalInput")
    iotap_d = nc.dram_tensor("iota_p", [P, 1], f32, kind="ExternalInput")
    iotaf_d = nc.dram_tensor("iota_f", [P, P], f32, kind="ExternalInput")
    ones_d = nc.dram_tensor("ones_row", [1, P], f32, kind="ExternalInput")
    onesc_d = nc.dram_tensor("ones_col", [P, 1], f32, kind="ExternalInput")
    wrap_d = nc.dram_tensor("wrap16", [P, 8], f32, kind="ExternalInput")

    lp_d = nc.dram_tensor("log_probs_s", [B * S, VS], f32, kind="ExternalOutput")
    probs_d = nc.dram_tensor("probs_o", [P, NCH], f32, kind="ExternalOutput")
    glog_d = nc.dram_tensor("glog_o", [P, NCH], f32, kind="ExternalOutput")
    g_d = nc.dram_tensor("g_o", [P, NCH], f32, kind="ExternalOutput")
    ddst_d = nc.dram_tensor("ddst_o", [P, NCH], f32, kind="ExternalOutput")
    udst_d = nc.dram_tensor("udst_o", [P, NCH], f32, kind="ExternalOutput")
    posd_d = nc.dram_tensor("posd_o", [P, NCH], f32, kind="ExternalOutput")

    # internal DRAM
    sums_d = [nc.dram_tensor(f"sums_r{r}", [S, AUGW], f32, kind="Internal")
              for r in range(B)]
    yt_d = [nc.dram_tensor(f"yt{t}", [DC, P, P], bf16, kind="Internal")
            for t in range(NCH)]
    ngrp = NCH // G
    arin_d = [nc.dram_tensor(f"arin{g}", [P, G], f32, kind="Internal")
              for g in range(ngrp)]
    arout_d = [nc.dram_tensor(f"arout{g}", [P, G], f32, kind="Internal",
                              addr_space="Shared") for g in range(ngrp)]

    # chunk-layout views of (B, S) tensors: [p, cc] with token = cc*P + p
    ids_pc_ap = ids_d.ap().rearrange("b (c p) -> p (b c)", p=P)
    noise_pc_ap = noise_d.ap().rearrange("b (c p) -> p (b c)", p=P)

    with tile.TileContext(nc) as tc:
        with (
            tc.tile_pool(name="const", bufs=1) as constp,
            tc.tile_pool(name="persist", bufs=1) as persp,
            tc.tile_pool(name="resident", bufs=1) as resp,
        ):
            # ---- resident weights/consts ----
            w_sb = resp.tile([P, DC * VS], bf16, tag="w_sb")
            for k in range(DC):
                nc.sync.dma_start(w_sb[:, k * VS:(k + 1) * VS], wsh_d.ap()[k])
            bias_sb = resp.tile([P, VS], bf16, tag="bias_sb")
            nc.sync.dma_start(bias_sb[:], bias_d.ap())
            tri_sb = constp.tile([P, P], f32, tag="tri")
            nc.sync.dma_start(tri_sb[:], tri_d.ap())
            tris_sb = constp.tile([NCH, NCH], f32, tag="tris")
            nc.sync.dma_start(tris_sb[:], tris_d.ap())
            ident_sb = constp.tile([P, P], f32, tag="ident")
            nc.sync.dma_start(ident_sb[:], ident_d.ap())
            iotap_sb = constp.tile([P, 1], f32, tag="iotap")
            nc.sync.dma_start(iotap_sb[:], iotap_d.ap())
            iotaf_sb = constp.tile([P, P], f32, tag="iotaf")
            nc.sync.dma_start(iotaf_sb[:], iotaf_d.ap())
            ones_sb = constp.tile([1, P], f32, tag="ones")
            nc.sync.dma_start(ones_sb[:], ones_d.ap())
            onesc_sb = constp.tile([P, 1], f32, tag="onesc")
            nc.sync.dma_start(onesc_sb[:], onesc_d.ap())
            wrap_sb = constp.tile([P, 8], f32, tag="wrap16")
            nc.sync.dma_start(wrap_sb[:], wrap_d.ap())
            gwb_sb = constp.tile([P, AUGW], f32, tag="gwb")
            nc.sync.dma_start(gwb_sb[:], gwb_d.ap())
            ids_sb = persp.tile([P, NCH], i32, tag="ids")
            nc.sync.dma_start(ids_sb[:], ids_pc_ap)
            noise_sb = persp.tile([P, NCH], f32, tag="noise")
            nc.sync.dma_start(noise_sb[:], noise_pc_ap)
            nt16 = B * S // 16
            ids16i = persp.tile([P, nt16], i32, tag="ids16i")
            for kq in range(8):
                nc.sync.dma_start(
                    ids16i[16 * kq:16 * (kq + 1), :],
                    ids_d.ap().rearrange("b (q p) -> p (b q)", p=16))
            ids16 = persp.tile([P, nt16], dt.int16, tag="ids16")
            nc.vector.tensor_copy(ids16[:], ids16i[:])

            # zero the pooled-sums scratch
            with tc.tile_pool(name="zero", bufs=1) as zp:
                zt = zp.tile([P, AUGW], f32, tag="zt")
                nc.vector.memset(zt[:], 0.0)
                for r in range(B):
                    for j in range(SC):
                        nc.sync.dma_start(
                            sums_d[r].ap()[j * P:(j + 1) * P, :], zt[:])

            # persistent small tiles
            glog = persp.tile([P, NCH], f32, tag="glog")
            g_f = persp.tile([P, NCH], f32, tag="g_f")
            probs = persp.tile([P, NCH], f32, tag="probs")
            incl = persp.tile([P, NCH], f32, tag="incl")
            cum = persp.tile([P, NCH], f32, tag="cum")       # incl + offs
            ddst = persp.tile([P, NCH], f32, tag="ddst")
            udst = persp.tile([P, NCH], f32, tag="udst")
            lexc = persp.tile([P, NCH], f32, tag="lexc")
            offsB = persp.tile([P, NCH], f32, tag="offsB")
            ubc = persp.tile([P, NCH], f32, tag="ubc")
            lu = persp.tile([P, NCH], f32, tag="lu")
            posd_all = persp.tile([P, NCH], f32, tag="posd")

            # ================= stage A: gather x, gate, cumsum =============
            with (
                tc.tile_pool(name="xg", bufs=3) as xgp,
                tc.tile_pool(name="sa", bufs=2) as sap,
                tc.tile_pool(name="psA", bufs=2, space="PSUM") as psA,
            ):
                GB = min(4, NCH)  # chunks per dma_gather batch
                for cb in range(0, NCH, GB):
                    x4 = xgp.tile([P, GB * D], f32, tag="x4")
                    nc.gpsimd.dma_gather(
                        x4[:].rearrange("p (g e) -> p g e", g=GB),
                        emb_d.ap(),
                        ids16[:, cb * 8:(cb + GB) * 8],
                        num_idxs=GB * P, num_idxs_reg=GB * P,
                        elem_size=D)
                    for j in range(GB):
                        cc = cb + j
                        tmp = sap.tile([P, D], f32, tag="gtmp")
                        nc.vector.tensor_tensor(
                            tmp[:], x4[:, j * D:(j + 1) * D],
                            gwb_sb[:, 0:D], ALU.mult)
                        nc.vector.tensor_reduce(
                            glog[:, cc:cc + 1], tmp[:],
                            mybir.AxisListType.X, ALU.add)
                # gate_b add, sigmoid, bernoulli
                nc.scalar.activation(probs[:], glog[:], AF.Sigmoid,
                                     bias=gwb_sb[:, D + 1:D + 2], scale=1.0)
                # glog output should include bias: glog += gate_b
                nc.vector.tensor_scalar(glog[:], glog[:],
                                        gwb_sb[:, D + 1:D + 2], None, ALU.add)
                nc.vector.tensor_tensor(g_f[:], noise_sb[:], probs[:],
                                        ALU.is_lt)
                for r in range(B):
                    nc.vector.memset(g_f[0:1, r * SC:r * SC + 1], 1.0)

                # cumsum within chunks
                incl_ps = psA.tile([P, NCH], f32, tag="ps_small")
                nc.tensor.matmul(incl_ps[:], tri_sb[:], g_f[:],
                                 start=True, stop=True)
                nc.vector.tensor_copy(incl[:], incl_ps[:])
                # chunk totals: sum of g over partitions -> [1, NCH]
                totR_ps = psA.tile([1, NCH], f32, tag="ps_small3")
                nc.tensor.matmul(totR_ps[:], onesc_sb[:], g_f[:],
                                 start=True, stop=True)
                totR = sap.tile([1, NCH], f32, tag="totR")
                nc.vector.tensor_copy(totR[:], totR_ps[:])
                totT_ps = psA.tile([NCH, 1], f32, tag="ps_small2")
                nc.tensor.matmul(totT_ps[:], totR[:],
                                 ones_sb[0:1, 0:1], start=True, stop=True)
                totT = sap.tile([NCH, 1], f32, tag="totT")
                nc.vector.tensor_copy(totT[:], totT_ps[:])
                # exclusive per-row chunk offsets [NCH, 1]
                offsT_ps = psA.tile([NCH, 1], f32, tag="ps_small2")
                nc.tensor.matmul(offsT_ps[:], tris_sb[:], totT[:],
                                 start=True, stop=True)
                offsT = sap.tile([NCH, 1], f32, tag="offsT")
                nc.vector.tensor_copy(offsT[:], offsT_ps[:])
                # transpose back to row [1, NCH]
                offsR_ps = psA.tile([1, NCH], f32, tag="ps_small2")
                nc.tensor.matmul(offsR_ps[:], offsT[:],
                                 ident_sb[0:NCH, 0:NCH], start=True, stop=True)
                offsR = sap.tile([1, NCH], f32, tag="offsR")
                nc.vector.tensor_copy(offsR[:], offsR_ps[:])
                # broadcast offs to all partitions
                offsB_ps = psA.tile([P, NCH], f32, tag="ps_small")
                nc.tensor.matmul(offsB_ps[:], ones_sb[:], offsR[:],
                                 start=True, stop=True)
                nc.vector.tensor_copy(offsB[:], offsB_ps[:])

                nc.vector.tensor_tensor(cum[:], incl[:], offsB[:], ALU.add)
                nc.vector.tensor_tensor(ddst[:], cum[:], g_f[:], ALU.subtract)
                nc.vector.tensor_scalar(udst[:], cum[:], 1.0, None,
                                        ALU.subtract)
                nc.vector.tensor_tensor(lexc[:], incl[:], g_f[:],
                                        ALU.subtract)
                # upsample window base: u_c = udst[0, c] broadcast
                ubc_ps = psA.tile([P, NCH], f32, tag="ps_small")
                nc.tensor.matmul(ubc_ps[:], ones_sb[:], udst[0:1, :],
                                 start=True, stop=True)
                nc.vector.tensor_copy(ubc[:], ubc_ps[:])
                nc.vector.tensor_tensor(lu[:], udst[:], ubc[:], ALU.subtract)

                # small outputs
                nc.sync.dma_start(probs_d.ap(), probs[:])
                nc.sync.dma_start(glog_d.ap(), glog[:])
                nc.sync.dma_start(g_d.ap(), g_f[:])
                nc.sync.dma_start(ddst_d.ap(), ddst[:])
                nc.sync.dma_start(udst_d.ap(), udst[:])

            # ================= stage B: pooled sums via scatter-add ========
            with (
                tc.tile_pool(name="xb", bufs=2) as xbp,
                tc.tile_pool(name="sb", bufs=3) as sbp,
                tc.tile_pool(name="idxp", bufs=4) as idxp,
                tc.tile_pool(name="psB", bufs=2, space="PSUM") as psB,
            ):
                GB = min(4, SC)
                for r in range(B):
                    for cb in range(0, SC, GB):
                        xb = xbp.tile([P, GB * D], f32, tag="x4")
                        c0 = r * SC + cb
                        nc.gpsimd.dma_gather(
                            xb[:].rearrange("p (g e) -> p g e", g=GB),
                            emb_d.ap(),
                            ids16[:, c0 * 8:(c0 + GB) * 8],
                            num_idxs=GB * P, num_idxs_reg=GB * P,
                            elem_size=D)
                        for j in range(GB):
                            cc = c0 + j
                            po = sbp.tile([P, 64], f32, tag="po")
                            nc.vector.tensor_scalar(
                                po[:, 0:1], iotap_sb[:],
                                float((cb + j) * P), None, ALU.add)
                            nc.vector.memset(po[:, 1:2], 1.0)
                            nc.vector.memset(po[:, 2:64], 0.0)
                            oh = sbp.tile([P, P], f32, tag="oh")
                            nc.vector.tensor_scalar(
                                oh[:], iotaf_sb[:], lexc[:, cc:cc + 1],
                                None, ALU.is_equal)
                            pp = psB.tile([P, AUGW], f32, tag="pp")
                            for (doff, dn) in ((0, 512), (512, D - 512)) \
                                    if D > 512 else ((0, D),):
                                nc.tensor.matmul(
                                    pp[:, doff:doff + dn], oh[:],
                                    xb[:, j * D + doff:j * D + doff + dn],
                                    start=True, stop=True)
                            nc.tensor.matmul(pp[:, D:D + 64], oh[:], po[:],
                                             start=True, stop=True)
                            pS = sbp.tile([P, AUGW], f32, tag="pS")
                            nc.vector.tensor_copy(pS[:], pp[:])
                            # scatter-add into global dst rows
                            idxf = idxp.tile([P, 8], f32, tag="idxf")
                            nc.vector.tensor_scalar(
                                idxf[:], wrap_sb[:], offsB[:, cc:cc + 1],
                                None, ALU.add)
                            idx16 = idxp.tile([P, 8], dt.int16, tag="idx16")
                            nc.vector.tensor_copy(idx16[:], idxf[:])
                            nc.gpsimd.dma_scatter_add(
                                sums_d[r].ap(),
                                pS[:].rearrange("p (g e) -> p g e", g=1),
                                idx16[:], num_idxs=P, num_idxs_reg=P,
                                elem_size=AUGW)

                # pos_down means from pooled cols [D], [D+1]
                for r in range(B):
                    for j in range(SC):
                        ps2 = sbp.tile([P, 2], f32, tag="ps2")
                        nc.sync.dma_start(
                            ps2[:], sums_d[r].ap()[j * P:(j + 1) * P,
                                                   D:D + 2])
                        mx = sbp.tile([P, 1], f32, tag="mx")
                        nc.vector.tensor_scalar(mx[:], ps2[:, 1:2], 1.0,
                                                None, ALU.max)
                        inv = sbp.tile([P, 1], f32, tag="invp")
                        nc.vector.reciprocal(inv[:], mx[:])
                        nc.vector.tensor_tensor(
                            posd_all[:, r * SC + j:r * SC + j + 1],
                            ps2[:, 0:1], inv[:], ALU.mult)
                nc.sync.dma_start(posd_d.ap(), posd_all[:])

            # ================= stage C: upsample + transpose -> yT =========
            with (
                tc.tile_pool(name="xc", bufs=3) as xcp,
                tc.tile_pool(name="sc", bufs=3) as scp,
                tc.tile_pool(name="yts", bufs=4) as ytsp,
                tc.tile_pool(name="psC", bufs=2, space="PSUM") as psC,
                tc.tile_pool(name="psY", bufs=4, space="PSUM") as psY,
            ):
                GB = min(2, SC)
                for r in range(B):
                    for cb in range(0, SC, GB):
                        c0 = r * SC + cb
                        xc = xcp.tile([P, GB * D], f32, tag="x4")
                        nc.gpsimd.dma_gather(
                            xc[:].rearrange("p (g e) -> p g e", g=GB),
                            emb_d.ap(),
                            ids16[:, c0 * 8:(c0 + GB) * 8],
                            num_idxs=GB * P, num_idxs_reg=GB * P,
                            elem_size=D)
                        win = scp.tile([P, GB * AUGW], f32, tag="win")
                        for j in range(GB):
                            idxf = scp.tile([P, 8], f32, tag="idxwf")
                            nc.vector.tensor_scalar(
                                idxf[:], wrap_sb[:],
                                ubc[:, c0 + j:c0 + j + 1],
                                None, ALU.add)
                            idx16 = scp.tile([P, 8], dt.int16, tag="idxw16")
                            nc.vector.tensor_copy(idx16[:], idxf[:])
                            nc.gpsimd.dma_gather(
                                win[:, j * AUGW:(j + 1) * AUGW].rearrange(
                                    "p (g e) -> p g e", g=1),
                                sums_d[r].ap(), idx16[:],
                                num_idxs=P, num_idxs_reg=P,
                                elem_size=AUGW)
                        for j in range(GB):
                            cc = r * SC + cb + j
                            # onehot [t, l] then PE-transpose to [l, t]
                            oht = scp.tile([P, P], f32, tag="oht")
                            nc.vector.tensor_scalar(
                                oht[:], iotaf_sb[:], lu[:, cc:cc + 1],
                                None, ALU.is_equal)
                            ohT_ps = psC.tile([P, P], f32, tag="ohT")
                            nc.tensor.transpose(ohT_ps[:], oht[:],
                                                ident_sb[:])
                            # scale rows by 1/max(cnt,1)
                            mx = scp.tile([P, 1], f32, tag="mxc")
                            nc.vector.tensor_scalar(
                                mx[:], win[:, j * AUGW + D + 1:
                                            j * AUGW + D + 2],
                                1.0, None, ALU.max)
                            inv = scp.tile([P, 1], f32, tag="inv")
                            nc.vector.reciprocal(inv[:], mx[:])
                            rhsU = scp.tile([P, P], f32, tag="rhsU")
                            nc.vector.tensor_scalar(rhsU[:], ohT_ps[:],
                                                    inv[:, 0:1], None,
                                                    ALU.mult)
                            for i in range(DC):
                                ytp = psY.tile([P, P], f32, tag="ytp")
                                nc.tensor.matmul(
                                    ytp[:],
                                    win[:, j * AUGW + i * P:
                                        j * AUGW + (i + 1) * P],
                                    rhsU[:], start=True, stop=False)
                                nc.tensor.matmul(
                                    ytp[:],
                                    xc[:, j * D + i * P:j * D + (i + 1) * P],
                                    ident_sb[:], start=False, stop=True)
                                yts = ytsp.tile([P, P], bf16, tag="yts")
                                nc.vector.tensor_copy(yts[:], ytp[:])
                                nc.sync.dma_start(yt_d[cc].ap()[i], yts[:])

            # ================= stage D: vocab matmul + log_softmax =========
            with (
                tc.tile_pool(name="ytk", bufs=3) as ytkp,
                tc.tile_pool(name="lgs", bufs=G + 1) as lgp,
                tc.tile_pool(name="expp", bufs=2) as expp,
                tc.tile_pool(name="finp", bufs=3) as finp,
                tc.tile_pool(name="dsm", bufs=3) as dsmp,
                tc.tile_pool(name="dram", bufs=2, space="DRAM") as drp,
                tc.tile_pool(name="psD", bufs=2, space="PSUM") as psD,
            ):
                nvt = len(c["VT"])
                half = (nvt + 1) // 2
                for grp in range(ngrp):
                    parts = dsmp.tile([P, G], f32, tag="parts")
                    logs = []
                    for j in range(G):
                        tt = grp * G + j
                        ytk = ytkp.tile([P, DC * P], bf16, tag="ytk")
                        # [k, p, t] loaded as [p, (k t)]
                        nc.sync.dma_start(
                            ytk[:].rearrange("p (k t) -> p k t", k=DC),
                            yt_d[tt].ap().rearrange("k p t -> p k t"))
                        lg = lgp.tile([P, VS], bf16, tag="lg")
                        logs.append(lg)
                        for hs in range(0, nvt, half):
                            vts = c["VT"][hs:hs + half]
                            w0 = vts[0][0]
                            wn = sum(n for _, n in vts)
                            ps = psD.tile([P, wn], f32, tag="psD")
                            for k in range(DC):
                                for (voff, vn) in vts:
                                    nc.tensor.matmul(
                                        ps[:, voff - w0:voff - w0 + vn],
                                        ytk[:, k * P:(k + 1) * P],
                                        w_sb[:, k * VS + voff:
                                             k * VS + voff + vn],
                                        start=(k == 0), stop=(k == DC - 1))
                            nc.vector.tensor_tensor(
                                lg[:, w0:w0 + wn], ps[:],
                                bias_sb[:, w0:w0 + wn], ALU.add)
                        ex = expp.tile([P, VS], bf16, tag="ex")
                        nc.scalar.activation(ex[:], lg[:], AF.Exp,
                                             accum_out=parts[:, j:j + 1])
                    # combine partial sums across cores
                    arin_t = arin_d[grp]
                    arout_t = arout_d[grp]
                    nc.sync.dma_start(arin_t.ap(), parts[:])
                    if with_collective:
                        nc.gpsimd.collective_compute(
                            "AllReduce", ALU.add,
                            replica_groups=[list(range(NCORES))],
                            ins=[arin_t.ap().opt()],
                            outs=[arout_t.ap().opt()],
                        )
                        gsum = dsmp.tile([P, G], f32, tag="gsum")
                        nc.sync.dma_start(gsum[:], arout_t.ap())
                    else:
                        gsum = dsmp.tile([P, G], f32, tag="gsum")
                        nc.sync.dma_start(gsum[:], arin_t.ap())
                    nlse = dsmp.tile([P, G], f32, tag="nlse")
                    nc.scalar.activation(nlse[:], gsum[:], AF.Ln)
                    nc.vector.tensor_scalar(nlse[:], nlse[:], -1.0, None,
                                            ALU.mult)
                    FH = VS // 2
                    for j in range(G):
                        tt = grp * G + j
                        for h in range(2):
                            fin = finp.tile([P, FH], f32, tag="fin")
                            nc.scalar.activation(
                                fin[:], logs[j][:, h * FH:(h + 1) * FH],
                                AF.Identity, bias=nlse[:, j:j + 1],
                                scale=1.0)
                            nc.sync.dma_start(
                                lp_d.ap()[tt * P:(tt + 1) * P,
                                          h * FH:(h + 1) * FH],
                                fin[:])
    nc.compile()
    return nc, c


# ---------------------------------------------------------------------------
# host side
# ---------------------------------------------------------------------------

def host_inputs(cfg, input_ids, gate_noise, emb, gate_w, gate_b, out_w,
                out_b):
    """Build per-core in_maps."""
    import ml_dtypes
    c = _derived(cfg)
    B, S, D, V, VS, NCH, AUGW = (c["B"], c["S"], c["D"], c["V"], c["VS"],
                                 c["NCH"], c["AUGW"])
    gwb = np.zeros((P, AUGW), np.float32)
    gwb[:, :D] = np.asarray(gate_w)[None, :]
    gwb[:, D + 1] = np.asarray(gate_b)[0]
    tri = np.tril(np.ones((P, P), np.float32)).T  # tri[p', p] = p' <= p
    # strict lower within each row's chunk block
    tris = np.zeros((NCH, NCH), np.float32)
    SCc = c["SC"]
    for r in range(B):
        for a in range(SCc):
            for b_ in range(a):
                tris[r * SCc + b_, r * SCc + a] = 1.0
    ident = np.eye(P, dtype=np.float32)
    iota_p = np.arange(P, dtype=np.float32)[:, None]
    iota_f = np.tile(np.arange(P, dtype=np.float32)[None, :], (P, 1))
    ones_row = np.ones((1, P), np.float32)

    common = dict(
        input_ids=np.ascontiguousarray(np.asarray(input_ids, np.int32)),
        gate_noise=np.ascontiguousarray(np.asarray(gate_noise, np.float32)),
        emb=np.ascontiguousarray(np.asarray(emb, np.float32)),
        gate_wb=gwb, tri128=np.ascontiguousarray(tri), tri_chunk=tris,
        ident=ident, iota_p=iota_p, iota_f=iota_f, ones_row=ones_row,
        ones_col=np.ones((P, 1), np.float32),
        wrap16=(np.tile(np.arange(16, dtype=np.float32), 8)[:, None]
                + 16.0 * np.arange(8, dtype=np.float32)[None, :]),
    )
    w = np.asarray(out_w, np.float32)
    ob = np.asarray(out_b, np.float32)
    in_maps = []
    for core in range(c["NCORES"]):
        m = dict(common)
        ws = w[:, core * VS:(core + 1) * VS].reshape(c["DC"], P, VS)
        m["w_shard"] = np.ascontiguousarray(ws.astype(ml_dtypes.bfloat16))
        m["bias_bc"] = np.ascontiguousarray(
            np.tile(ob[None, core * VS:(core + 1) * VS], (P, 1))
            .astype(ml_dtypes.bfloat16))
        in_maps.append(m)
    return in_maps


def assemble(cfg, results):
    """Convert per-core result dicts to the reference output tuple."""
    c = _derived(cfg)
    B, S, VS, NCH = c["B"], c["S"], c["VS"], c["NCH"]
    r0 = results[0]

    def pc_to_bs(a):  # [P, NCH] -> (B, S)
        return np.ascontiguousarray(a.T.reshape(B, S))

    lp = np.concatenate([results[i]["log_probs_s"] for i in
                         range(c["NCORES"])], axis=1)
    log_probs = lp.reshape(B, S, c["V"])
    probs = pc_to_bs(r0["probs_o"])
    glog = pc_to_bs(r0["glog_o"])
    g = pc_to_bs(r0["g_o"]).astype(np.int64)
    ddst = pc_to_bs(r0["ddst_o"]).astype(np.int32)
    udst = pc_to_bs(r0["udst_o"]).astype(np.int32)
    n_dst = (ddst[:, -1] + 1).astype(np.int32)
    # posd: [P, NCH] col cc = r*SC + j, partition p -> dst j*P+p of row r
    pd = r0["posd_o"]
    SCc = c["SC"]
    pos_down = np.zeros((B, S), np.float32)
    for r in range(B):
        pos_down[r] = pd[:, r * SCc:(r + 1) * SCc].T.reshape(S)
    return (log_probs, probs, glog, g, ddst, udst, n_dst, pos_down)


_CACHE = {}


def _get_nc():
    if "nc" not in _CACHE:
        cfg = full_cfg()
        nc, c = build_nc(cfg)
        _CACHE["nc"] = nc
        _CACHE["cfg"] = cfg
    return _CACHE["nc"], _CACHE["cfg"]


def kernel(input_ids, gate_noise, emb, gate_w, gate_b, out_w, out_b):
    nc, cfg = _get_nc()
    in_maps = host_inputs(cfg, input_ids, gate_noise, emb, gate_w, gate_b,
                          out_w, out_b)
    results = bass2jax.run_bass_via_pjrt(nc, in_maps,
                                         n_cores=cfg["NCORES"])
    return assemble(cfg, results)
